# revision 118
# baseline (speedup 1.0000x reference)
"""Spiking self-attention (SpikFormer SSA) on 8 TRN2 cores — v4.

vs v2 baseline (371us):
  - qkv/proj weights as fp16 hi+lo dual planes (exact to ~22 bits, both
    planes accumulate into the same PSUM bank — no nibble B-bank merge).
  - LIF1 runs in transposed [c, n] layout (x^T via PE): its hard-reset mask
    m = (1-spike)/2 doubles as the qkv matmul operand; BN shift-invariance
    absorbs the 1-2m substitution (host negates gamma, eps -> eps/4).
    Same trick for the LIF-proj mask feeding the proj matmul.  This deletes
    the spike-transpose pipeline and two elementwise passes per timestep.
  - potentials fp32 (flip-exact vs the fp32 reference: rel err ~1.6e-5);
    spikes/masks/attention counts fp16 (exact values).
  - half-group (2-timestep) AllReduce granularity, flat slot schedule with
    prefetched x DMAs and BN1-apply, stores via SP queue, weight loads on
    the scalar queue.  bn_stats halves are element-INTERLEAVED; merge is
    sum/sumsq symmetric.
"""

import numpy as np

import concourse.bass as bass
import concourse.bacc as bacc
import concourse.tile as tile
from concourse import mybir, masks
from concourse import bass_utils
from concourse.mybir import AluOpType as op
from concourse.mybir import ActivationFunctionType as act

F32 = mybir.dt.float32
F16 = mybir.dt.float16

T, B, N, C = 16, 8, 196, 512
H = 8
O3 = 3 * C
NT0, NT1 = 128, N - 128
KT = C // 128          # 4 k-tiles
OT1 = O3 // 128        # 12
OT2 = C // 128         # 4
NB = B * N
EPS = 1e-5
GT = 4
NG = T // GT
N_CORES = 8


def _build(sim_mode=False, debug=False):
    nc = bacc.Bacc("TRN2", target_bir_lowering=False, debug=False,
                   num_devices=1 if sim_mode else N_CORES)

    x_d = nc.dram_tensor("x", [T, N, C], F32, kind="ExternalInput").ap()
    wq_d = nc.dram_tensor("wq", [2 * KT, 128, O3], F16, kind="ExternalInput").ap()
    wp_d = nc.dram_tensor("wp", [KT, 128, C], F16, kind="ExternalInput").ap()
    g1_d = nc.dram_tensor("g1", [128, OT1 * T], F32, kind="ExternalInput").ap()
    b1_d = nc.dram_tensor("b1", [128, OT1 * T], F32, kind="ExternalInput").ap()
    g2_d = nc.dram_tensor("g2", [128, OT2 * T], F32, kind="ExternalInput").ap()
    b2_d = nc.dram_tensor("b2", [128, OT2 * T], F32, kind="ExternalInput").ap()
    out_d = nc.dram_tensor("out", [T, N, C], F32, kind="ExternalOutput").ap()
    if debug:
        dbg_m = nc.dram_tensor("dbg_m", [128, KT * N], F16, kind="ExternalOutput").ap()
        dbg_y = nc.dram_tensor("dbg_y", [128, OT1 * N], F32, kind="ExternalOutput").ap()
        dbg_sc = nc.dram_tensor("dbg_sc", [128, OT1 * T], F32, kind="ExternalOutput").ap()
        dbg_bi = nc.dram_tensor("dbg_bi", [128, OT1 * T], F32, kind="ExternalOutput").ap()
        dbg_st = nc.dram_tensor("dbg_st", [128, 6 * OT1 * T], F32, kind="ExternalOutput").ap()
        dbg_yn = nc.dram_tensor("dbg_yn", [128, OT1 * N], F32, kind="ExternalOutput").ap()
        dbg_sT = nc.dram_tensor("dbg_sT", [128, OT1 * N], F16, kind="ExternalOutput").ap()
        dbg_att = nc.dram_tensor("dbg_att", [128, OT2 * N], F16, kind="ExternalOutput").ap()
        dbg_pot = nc.dram_tensor("dbg_pot", [128, OT2 * N], F32, kind="ExternalOutput").ap()

    with tile.TileContext(nc) as tc:
        import contextlib
        stack = contextlib.ExitStack()
        const = stack.enter_context(tc.tile_pool(name="const", bufs=1))
        state = stack.enter_context(tc.tile_pool(name="state", bufs=1))
        work = stack.enter_context(tc.tile_pool(name="work", bufs=2))
        ypool = stack.enter_context(tc.tile_pool(name="ypool", bufs=5))
        popool = stack.enter_context(tc.tile_pool(name="popool", bufs=4))
        psum = stack.enter_context(tc.tile_pool(name="psum", bufs=1, space="PSUM"))
        psum3 = stack.enter_context(tc.tile_pool(name="psum3", bufs=3, space="PSUM"))
        dram = stack.enter_context(tc.tile_pool(name="dram", bufs=1, space="DRAM"))

        def mmA():
            return psum3.tile([128, 2, 256], F32, tag="mmA", name="mmA")

        def o2d():
            return psum.tile([128, 512], F32, tag="o2d", name="o2d", bufs=2)

        ident = const.tile([128, 128], F16, tag="id16", name="ident")
        masks.make_identity(nc, ident[:])
        identf = const.tile([128, 128], F32, tag="id32", name="identf")
        masks.make_identity(nc, identf[:])

        nsl = [(0, NT0), (NT0, NT1)]

        # first x tiles load BEFORE weights: the SP queue + HWDGE serialize,
        # and LIF1(t=0) needs x immediately while matmuls need weights ~10us in
        xbufs = {}

        def load_x(t, eng=None):
            xs = [work.tile([128, C], F32, tag=f"x{i}", name=f"x{i}", bufs=3)
                  for i in range(2)]
            for i, (o, sz) in enumerate(nsl):
                (eng or nc.sync).dma_start(xs[i][:sz, :], x_d[t, o:o + sz, :])
            return xs

        xbufs[0] = load_x(0)
        xbufs[1] = load_x(1)

        wq = const.tile([128, 2 * KT, O3], F16, tag="wq", name="wq")
        wp = const.tile([128, KT, C], F16, tag="wp", name="wp")
        for k in range(2 * KT):
            nc.scalar.dma_start(wq[:, k, :], wq_d[k])
            if k < KT:
                nc.scalar.dma_start(wp[:, k, :], wp_d[k])

        g1 = const.tile([128, OT1 * T], F32, tag="g1", name="g1")
        b1 = const.tile([128, OT1 * T], F32, tag="b1", name="b1")
        g2 = const.tile([128, OT2 * T], F32, tag="g2", name="g2")
        b2 = const.tile([128, OT2 * T], F32, tag="b2", name="b2")
        for t_ap, d_ap in [(g1, g1_d), (b1, b1_d), (g2, g2_d), (b2, b2_d)]:
            nc.scalar.dma_start(t_ap[:], d_ap[:, :])

        # LIF state in natural units r = v_post: w = r + x (tt, 2x mode),
        # spike = w >= 2*VTH (ts, 4x), reset r = w * ((w < 2*VTH)*0.5)
        # (mask via ts 4x, apply via tt 2x) — no scalar_tensor_tensor
        # anywhere on the hot path (it has no DVE fast modes).
        vd1 = [state.tile([128, 2 * N], F32, tag=f"vd1_{i}", name=f"vd1_{i}")
               for i in range(2)]
        vd2 = state.tile([128, OT1 * N], F32, tag="vd2", name="vd2")
        vd4 = state.tile([128, OT2, N], F32, tag="vd4", name="vd4")

        # k/v spike transposes land here: cols 0:C = k^T, C:2C = v^T
        def kvT_t():
            return [work.tile([128, 2 * C], F16, tag=f"kvT{i}", name=f"kvT{i}",
                              bufs=2) for i in range(2)]
        # bn params (written per group)
        sc1 = state.tile([128, OT1 * T], F32, tag="sc1", name="sc1")
        bi1 = state.tile([128, OT1 * T], F32, tag="bi1", name="bi1")
        sc2 = state.tile([128, OT2 * T], F32, tag="sc2", name="sc2")
        bi2 = state.tile([128, OT2 * T], F32, tag="bi2", name="bi2")
        # raw bn_stats: 6 cols per (t, ot) = (c,m,M2)x2 INTERLEAVED halves
        # (even/odd element index) — merge is still sum/sumsq symmetric
        st1 = state.tile([128, 6 * OT1 * T], F32, tag="st1", name="st1")
        st2 = state.tile([128, 6 * OT2 * T], F32, tag="st2", name="st2")
        for s in (vd1[0], vd1[1], vd2, vd4):
            nc.gpsimd.memset(s[:], 0.0)

        # half-group (2-timestep) allreduce granularity
        NH = T // 2
        W1 = OT1 * 2
        W2 = OT2 * 2
        ar1_in = [dram.tile([128, 2 * W1], F32, tag=f"a1i{h}", name=f"a1i{h}") for h in range(NH)]
        ar1_out = [dram.tile([128, 2 * W1], F32, tag=f"a1o{h}", name=f"a1o{h}") for h in range(NH)]
        ar2_in = [dram.tile([128, 2 * W2], F32, tag=f"a2i{h}", name=f"a2i{h}") for h in range(NH)]
        ar2_out = [dram.tile([128, 2 * W2], F32, tag=f"a2o{h}", name=f"a2o{h}") for h in range(NH)]
        # last half-group: per-single-timestep AR2 windows (tail latency)
        ar2s_in = {t: dram.tile([128, 2 * OT2], F32, tag=f"a2si{t}", name=f"a2si{t}")
                   for t in (T - 2, T - 1)}
        ar2s_out = {t: dram.tile([128, 2 * OT2], F32, tag=f"a2so{t}", name=f"a2so{t}")
                    for t in (T - 2, T - 1)}


        # ---------------- phase A ----------------
        def do_A(t):
            xs = xbufs[t]

            # transpose x into [c, n] layout (fp32 PE transpose, two
            # psum half-fills), then LIF1 entirely on DVE in that layout.
            # The LIF mask m = (w < 2VTH)*0.5 doubles as BOTH the hard-reset
            # multiplier AND the matmul operand: the matmul consumes
            # m = (1-spike)/2 instead of the spike, which BN absorbs
            # (shift-invariant; host folds g -> -g, eps -> eps/4).
            m1t = work.tile([128, KT, N], F16, tag="m1t", name="m1t")
            for hf in range(2):
                tpx = psum.tile([128, 2, N], F32, tag="tpx", name="tpx")
                for ci in range(2):
                    ct = 2 * hf + ci
                    for i, (o, sz) in enumerate(nsl):
                        nc.tensor.transpose(tpx[:, ci, o:o + sz],
                                            xs[i][:sz, ct * 128:(ct + 1) * 128],
                                            identf[:sz, :sz])
                vsl = vd1[hf]
                w1 = work.tile([128, 2 * N], F32, tag=f"l1w{hf}",
                               name=f"l1w{hf}", bufs=1)
                nc.vector.scalar_tensor_tensor(w1[:, :], vsl[:, :], 0.5,
                                               tpx[:, :, :], op.mult, op.add)
                nc.vector.tensor_scalar(m1t[:, 2 * hf:2 * hf + 2, :], w1[:, :],
                                        1.0, 0.5, op.is_lt, op.mult)
                nc.vector.scalar_tensor_tensor(vsl[:, :], w1[:, :], 1.0,
                                               w1[:, :], op.is_lt, op.mult)

            # qkv matmuls: fp16 weights, 4 k-tiles accumulate per ot;
            # 4 ot per double-bank fill, one act evac + two bn_stats each
            yt = ypool.tile([128, OT1, N], F32, tag="y", name=f"y{t}")
            for bk in range(OT1 // 2):
                pA = mmA()
                for q in range(2):
                    ot = 2 * bk + q
                    for k in range(2 * KT):
                        nc.tensor.matmul(pA[:, q, 0:N],
                                         wq[:, k, ot * 128:(ot + 1) * 128],
                                         m1t[:, k % KT, :],
                                         start=(k == 0), stop=(k == 2 * KT - 1))
                ysl = yt[:, 2 * bk:2 * bk + 2, :]
                nc.scalar.activation(ysl, pA[:, :, 0:N], act.Copy)
                for q in range(2):
                    ot = 2 * bk + q
                    scol = (t * OT1 + ot) * 6
                    nc.vector.bn_stats(st1[:, scol:scol + 6], yt[:, ot, :])
            if debug and t == 0:
                nc.sync.dma_start(dbg_m[:, :], m1t[:, :, :])
                nc.sync.dma_start(dbg_y[:, :], yt[:, :, :])
            return yt

        # ---------------- collectives + params ----------------
        def stage_stats(st, h2, w, stg, tmpw):
            # merge the two interleaved halves per (t,ot):
            # sum = h*(m0+m1) ; sumsq = M2_0+M2_1 + h*(m0^2+m1^2), h = N/2
            base = h2 * w * 6
            end = (h2 + 1) * w * 6
            m0 = st[:, base + 1: end: 6]
            M20 = st[:, base + 2: end: 6]
            m1 = st[:, base + 4: end: 6]
            M21 = st[:, base + 5: end: 6]
            h = float(N // 2)
            nc.vector.tensor_tensor(tmpw[:, 0:w], m0, m1, op.add)
            nc.vector.tensor_scalar(stg[:, 0:w], tmpw[:, 0:w], h, None, op.mult)
            nc.vector.tensor_tensor(tmpw[:, 0:w], m0, m0, op.mult)
            nc.vector.tensor_tensor(tmpw[:, w:2 * w], m1, m1, op.mult)
            nc.vector.tensor_tensor(tmpw[:, 0:w], tmpw[:, 0:w], tmpw[:, w:2 * w], op.add)
            nc.vector.tensor_scalar(tmpw[:, 0:w], tmpw[:, 0:w], h, None, op.mult)
            nc.vector.tensor_tensor(tmpw[:, 0:w], tmpw[:, 0:w], M20, op.add)
            nc.vector.tensor_tensor(stg[:, w:2 * w], tmpw[:, 0:w], M21, op.add)

        def ar(h2, st, w, arin, arout):
            stg = const.tile([128, 2 * w], F32, tag=f"stg{w}", name=f"stg{w}", bufs=2)
            tmpw = const.tile([128, 2 * w], F32, tag=f"stgt{w}", name=f"stgt{w}", bufs=2)
            stage_stats(st, h2, w, stg, tmpw)
            nc.sync.dma_start(arin[h2][:, :], stg[:, :])
            if sim_mode:
                nc.sync.dma_start(arout[h2][:], arin[h2][:])
            else:
                nc.gpsimd.collective_compute(
                    "AllReduce", op.add,
                    ins=[arin[h2].opt()], outs=[arout[h2].opt()],
                    replica_groups=[list(range(N_CORES))])

        def params(h2, w, arout, g_t, b_t, sc, bi, pfx):
            # w cols; arout: sums [0:w], sumsq [w:2w]
            gsum = const.tile([128, 2 * w], F32, tag=f"{pfx}gs", name=f"{pfx}gs", bufs=2)
            nc.sync.dma_start(gsum[:], arout[h2][:])
            cs = slice(h2 * w, (h2 + 1) * w)
            mean = const.tile([128, w], F32, tag=f"{pfx}mu", name=f"{pfx}mu", bufs=2)
            e2p = const.tile([128, w], F32, tag=f"{pfx}e2", name=f"{pfx}e2", bufs=2)
            rs = const.tile([128, w], F32, tag=f"{pfx}rs", name=f"{pfx}rs", bufs=2)
            tmp = const.tile([128, w], F32, tag=f"{pfx}t1", name=f"{pfx}t1", bufs=2)
            tmp2 = const.tile([128, w], F32, tag=f"{pfx}t2", name=f"{pfx}t2", bufs=2)
            # stats are of y' = W^T m with m = (1-s)/2; y = c - 2y' means
            # var_y = 4 var' — handled by eps/4 + host-negated gammas.
            nc.vector.tensor_scalar(mean[:], gsum[:, 0:w], 1.0 / NB, None, op.mult)
            nc.vector.tensor_scalar(tmp[:], gsum[:, w:2 * w], 1.0 / NB, EPS / 4.0,
                                    op.mult, op.add)
            nc.vector.tensor_tensor(tmp2[:], mean[:], mean[:], op.mult)
            nc.vector.tensor_tensor(e2p[:], tmp[:], tmp2[:], op.subtract)  # var+eps
            nc.vector.reciprocal(tmp[:], e2p[:])
            nc.scalar.activation(rs[:], tmp[:], act.Sqrt)
            for _ in range(1):  # Newton: rs *= 1.5 - 0.5*(var+eps)*rs^2
                nc.vector.tensor_tensor(tmp[:], rs[:], rs[:], op.mult)
                nc.vector.tensor_tensor(tmp2[:], tmp[:], e2p[:], op.mult)
                nc.vector.tensor_scalar(tmp[:], tmp2[:], -0.5, 1.5, op.mult, op.add)
                nc.vector.tensor_tensor(rs[:], rs[:], tmp[:], op.mult)
            nc.vector.tensor_tensor(sc[:, cs], rs[:], g_t[:, cs], op.mult)
            nc.vector.tensor_tensor(tmp[:], mean[:], sc[:, cs], op.mult)
            nc.vector.tensor_tensor(bi[:, cs], b_t[:, cs], tmp[:], op.subtract)

        # ---------------- phase B ----------------
        def do_B_yn(t):
            # BN1 apply on Pool (per-partition scale+bias pointers),
            # prefetched one slot ahead of the LIF2 consumers
            yt = ybufs[t]
            yn = work.tile([128, OT1, N], F32, tag="yn", name=f"yn{t}", bufs=3)
            for q3 in (1, 2, 0):
                for oi in range(4):
                    ot = q3 * 4 + oi
                    col = t * OT1 + ot
                    eng = nc.scalar if q3 == 1 else nc.gpsimd
                    if True:
                        nc.scalar.activation(yn[:, ot, :], yt[:, ot, :],
                                             act.Identity,
                                             bias=bi1[:, col:col + 1],
                                             scale=sc1[:, col:col + 1])
                    else:
                        nc.gpsimd.tensor_scalar(yn[:, ot, :], yt[:, ot, :],
                                                sc1[:, col:col + 1],
                                                bi1[:, col:col + 1],
                                                op.mult, op.add)
            return yn

        def do_B(t):
            yn = ynbufs[t]
            sT = work.tile([128, OT1 * N], F16, tag="sT", name="sT")
            for q3 in (1, 2, 0):  # k/v first: attention deps resolve early
                # LIF2 on DVE: w = r + yn (tt 2x); spike/mask ts 4x; reset tt
                sl = slice(q3 * 4 * N, (q3 + 1) * 4 * N)
                ysl = yn[:, q3 * 4:(q3 + 1) * 4, :]
                w2 = work.tile([128, 4 * N], F32, tag="w2B", name="w2B", bufs=2)
                nc.vector.scalar_tensor_tensor(w2[:], vd2[:, sl], 0.5,
                                               ysl, op.mult, op.add)
                nc.gpsimd.tensor_scalar(sT[:, sl], w2[:], 1.0, None, op.is_ge)
                nc.vector.scalar_tensor_tensor(vd2[:, sl], w2[:], 1.0,
                                               w2[:], op.is_lt, op.mult)

            if debug and t == 0:
                nc.sync.dma_start(dbg_yn[:, :], ynbufs[t][:, :, :])
                nc.sync.dma_start(dbg_sT[:, :], sT[:, :])

            # attention: k/v transposes into one psum bank per n-slice
            kvT = kvT_t()
            for i, (o, sz) in enumerate(nsl):
                tp2 = psum.tile([128, 2, C], F16, tag="tp2", name="tp2")
                for j in (1, 2):  # k -> cols 0:C, v -> cols C:2C
                    for ci in range(4):
                        otg = 4 * j + ci
                        nc.tensor.transpose(tp2[:sz, j - 1, ci * 128:(ci + 1) * 128],
                                            sT[:, otg * N + o: otg * N + o + sz],
                                            ident[:128, :128])
                nc.scalar.activation(kvT[i][:sz, :], tp2[:sz, :, :], act.Copy)

            kvp = o2d()[:, 0:256]
            for ct in range(4):
                for hh in range(2):
                    h = 2 * ct + hh
                    off = hh * 64
                    hc = h * 64
                    nc.tensor.matmul(kvp[off:off + 64, ct * 64:(ct + 1) * 64],
                                     kvT[0][:, hc:hc + 64],
                                     kvT[0][:, C + hc:C + hc + 64],
                                     start=True, stop=False,
                                     tile_position=(0, off))
                    nc.tensor.matmul(kvp[off:off + 64, ct * 64:(ct + 1) * 64],
                                     kvT[1][:NT1, hc:hc + 64],
                                     kvT[1][:NT1, C + hc:C + hc + 64],
                                     start=False, stop=True,
                                     tile_position=(0, off))
            kv = work.tile([128, 256], F16, tag="kv", name="kv", bufs=1)
            nc.scalar.activation(kv[:, :], kvp[:, :], act.Copy)

            # q@kv into psum, evac to att f16 (integer counts: exact)
            att = work.tile([128, OT2, N], F16, tag="att", name="att", bufs=1)
            for cp in range(2):
                outp = o2d()[:, 0:2 * N]
                for q in range(2):
                    ct = 2 * cp + q
                    for hh in range(2):
                        off = hh * 64
                        nc.tensor.matmul(outp[off:off + 64, q * N:(q + 1) * N],
                                         kv[off:off + 64, ct * 64:(ct + 1) * 64],
                                         sT[off:off + 64, ct * N:(ct + 1) * N],
                                         start=True, stop=True,
                                         tile_position=(off, off))
                nc.scalar.activation(att[:, 2 * cp:2 * cp + 2, :], outp, act.Copy)

            # LIF-proj on DVE (state x8: w = r + att, threshold 8); the mask
            # m4 = (w<8)*0.5 is both the reset multiplier and the proj
            # matmul operand (BN2 absorbs the 1-2m substitution)
            w4 = work.tile([128, OT2, N], F32, tag="w4", name="w4", bufs=1)
            m4 = work.tile([128, KT, N], F16, tag="m4", name="m4", bufs=2)
            nc.vector.scalar_tensor_tensor(w4[:, :, :], vd4[:, :, :], 0.5,
                                           att[:, :, :], op.mult, op.add)
            nc.vector.tensor_scalar(m4[:, :, :], w4[:, :, :], 8.0, 0.5,
                                    op.is_lt, op.mult)
            nc.vector.scalar_tensor_tensor(vd4[:, :, :], w4[:, :, :], 8.0,
                                           w4[:, :, :], op.is_lt, op.mult)

            # proj matmuls: all 4 ot in one double-bank fill
            pot = popool.tile([128, OT2, N], F32, tag="po", name=f"po{t}")
            for bk in range(OT2 // 2):
                pP = mmA()
                for q in range(2):
                    ot = 2 * bk + q
                    for k in range(KT):
                        nc.tensor.matmul(pP[:, q, 0:N],
                                         wp[:, k, ot * 128:(ot + 1) * 128],
                                         m4[:, k, :],
                                         start=(k == 0), stop=(k == KT - 1))
                nc.scalar.activation(pot[:, 2 * bk:2 * bk + 2, :], pP[:, :, 0:N],
                                     act.Copy)
            for ot in range(OT2):
                scol = (t * OT2 + ot) * 6
                nc.vector.bn_stats(st2[:, scol:scol + 6], pot[:, ot, :])
            if debug and t == 0:
                nc.sync.dma_start(dbg_att[:, :], att[:, :, :])
                nc.sync.dma_start(dbg_pot[:, :], pot[:, :, :])
            return pot

        # ---------------- phase C ----------------
        def do_C(t):
            pot = pobufs[t]
            fin = work.tile([128, OT2, N], F32, tag="fin", name="fin")
            for ot in range(OT2):
                col = t * OT2 + ot
                nc.scalar.activation(fin[:, ot, :], pot[:, ot, :],
                                     act.Identity,
                                     bias=bi2[:, col:col + 1],
                                     scale=sc2[:, col:col + 1])
            for i, (o, sz) in enumerate(nsl):
                tpf = psum.tile([128, C], F32, tag="ftp", name="ftp")
                for ot in range(OT2):
                    nc.tensor.transpose(tpf[:sz, ot * 128:(ot + 1) * 128],
                                        fin[:, ot, o:o + sz],
                                        identf[:128, :128])
                fout = work.tile([128, C], F32, tag=f"fo{i}", name=f"fo{i}", bufs=2)
                nc.scalar.activation(fout[:sz, :], tpf[:sz, :], act.Copy)
                nc.sync.dma_start(out_d[t, o:o + sz, :], fout[:sz, :])

        # ---------------- pipelined emission (flat slot schedule) ----------
        # per timestep-slot s: A(s); ar1 after A(2h+1); params1 one slot on;
        # yn(t) prefetched at t+YL; B(t) at t+BL; ar2 right after B(2h+1);
        # params2 next slot; C(t) at t+CL.
        # B-work and yn are enqueued BEFORE A(s) so the latency-critical
        # attention/LIF chains sit ahead of A's bulk in the engine FIFOs;
        # x DMA is prefetched one slot ahead.
        BL, CL = 8, 10
        YL = BL - 2
        ybufs = {}
        ynbufs = {}
        pobufs = {}
        xbufs[0] = load_x(0)
        for s in range(T + CL + 1):
            if 2 <= s + 1 < T:
                xbufs[s + 1] = load_x(s + 1)

            if s >= 2 and (s - 2) % 2 == 0 and (s - 2) // 2 < NH:
                ar((s - 2) // 2, st1, W1, ar1_in, ar1_out)
            if s >= 3 and (s - 3) % 2 == 0 and (s - 3) // 2 < NH:
                params((s - 3) // 2, W1, ar1_out, g1, b1, sc1, bi1, "p1")
            if 0 <= s - YL < T:
                ynbufs[s - YL] = do_B_yn(s - YL)
            if 0 <= s - BL < T:
                pobufs[s - BL] = do_B(s - BL)
            hs = s - BL
            if hs >= 1 and (hs - 1) % 2 == 0 and (hs - 1) // 2 < NH - 1:
                ar((hs - 1) // 2, st2, W2, ar2_in, ar2_out)
            if hs >= 2 and (hs - 2) % 2 == 0 and (hs - 2) // 2 < NH - 1:
                params((hs - 2) // 2, W2, ar2_out, g2, b2, sc2, bi2, "p2")
            # last two timesteps: single-t AR2 right after each B
            if hs in (T - 2, T - 1):
                ar(hs, st2, OT2, ar2s_in, ar2s_out)
            if hs - 1 in (T - 2, T - 1):
                params(hs - 1, OT2, ar2s_out, g2, b2, sc2, bi2, "p2s")
            if s < T:
                ybufs[s] = do_A(s)
            if 0 <= s - CL < T:
                do_C(s - CL)

        if debug:
            nc.sync.dma_start(dbg_sc[:, :], sc1[:, :])
            nc.sync.dma_start(dbg_bi[:, :], bi1[:, :])
            nc.sync.dma_start(dbg_st[:, :], st1[:, :])

        stack.close()

    nc.compile()
    return nc


# ---------------- host-side prep ----------------

def _bn_layout(v, Tn, OT):
    return np.ascontiguousarray(
        np.asarray(v, np.float32).reshape(Tn, OT, 128)
        .transpose(2, 0, 1).reshape(128, OT * Tn))


def _prep(inputs):
    qkv_w = np.asarray(inputs["qkv_w"], dtype=np.float32)
    proj_w = np.asarray(inputs["proj_w"], dtype=np.float32)
    w1t = np.ascontiguousarray(qkv_w.T)   # [512, 1536]
    w2t = np.ascontiguousarray(proj_w.T)  # [512, 512]
    def hilo(w, M):
        hi = w.astype(np.float16)
        lo = (w - hi.astype(np.float32)).astype(np.float16)
        return np.concatenate([hi.reshape(KT, 128, M), lo.reshape(KT, 128, M)], axis=0)
    wq = hilo(w1t, O3)
    # proj has no downstream thresholds: its quantization error is smooth
    # (no spike-flip cascade), so a single fp16 plane suffices (L2 ~2.6e-4)
    wp = w2t.reshape(KT, 128, C).astype(np.float16)

    # matmuls consume masks m = (1-spike)/2, so y = c - 2*y' and BN needs
    # sc_eff = -g * rsqrt(var'+eps/4), bi_eff = b - mean' * sc_eff: negate g.
    g1 = -_bn_layout(inputs["bn1_g"], T, OT1)
    b1 = _bn_layout(inputs["bn1_b"], T, OT1)
    g2 = -_bn_layout(inputs["bn2_g"], T, OT2)
    b2 = _bn_layout(inputs["bn2_b"], T, OT2)
    return dict(wq=wq, wp=wp, g1=g1, b1=b1, g2=g2, b2=b2)


_CACHE = {}


def kernel(_trace=False, **inputs):
    for k in ("w_in", "w_q", "w_k", "w_v", "w_proj"):
        assert float(np.asarray(inputs[k])) == 0.0, "kernel assumes sigmoid(w)=0.5"
    if "nc" not in _CACHE:
        _CACHE["nc"] = _build()
    nc = _CACHE["nc"]

    shared = _prep(inputs)
    x = np.asarray(inputs["x"], dtype=np.float32)
    in_maps = []
    for b in range(N_CORES):
        m = dict(shared)
        m["x"] = np.ascontiguousarray(x[:, b])
        in_maps.append(m)

    res = bass_utils.run_bass_kernel_spmd(nc, in_maps, core_ids=list(range(N_CORES)),
                                          trace=_trace)
    out = np.stack([r["out"] for r in res.results], axis=1)
    if _trace:
        return out, res
    return out


# revision 119
# speedup vs baseline: 1.0339x; 1.0339x over previous
"""Spiking self-attention (SpikFormer SSA) on 8 TRN2 cores — v4.

vs v2 baseline (371us):
  - qkv/proj weights as fp16 hi+lo dual planes (exact to ~22 bits, both
    planes accumulate into the same PSUM bank — no nibble B-bank merge).
  - LIF1 runs in transposed [c, n] layout (x^T via PE): its hard-reset mask
    m = (1-spike)/2 doubles as the qkv matmul operand; BN shift-invariance
    absorbs the 1-2m substitution (host negates gamma, eps -> eps/4).
    Same trick for the LIF-proj mask feeding the proj matmul.  This deletes
    the spike-transpose pipeline and two elementwise passes per timestep.
  - potentials fp32 (flip-exact vs the fp32 reference: rel err ~1.6e-5);
    spikes/masks/attention counts fp16 (exact values).
  - half-group (2-timestep) AllReduce granularity, flat slot schedule with
    prefetched x DMAs and BN1-apply, stores via SP queue, weight loads on
    the scalar queue.  bn_stats halves are element-INTERLEAVED; merge is
    sum/sumsq symmetric.
"""

import numpy as np

import concourse.bass as bass
import concourse.bacc as bacc
import concourse.tile as tile
from concourse import mybir, masks
from concourse import bass_utils
from concourse.mybir import AluOpType as op
from concourse.mybir import ActivationFunctionType as act

F32 = mybir.dt.float32
F16 = mybir.dt.float16

T, B, N, C = 16, 8, 196, 512
H = 8
O3 = 3 * C
NT0, NT1 = 128, N - 128
KT = C // 128          # 4 k-tiles
OT1 = O3 // 128        # 12
OT2 = C // 128         # 4
NB = B * N
EPS = 1e-5
GT = 4
NG = T // GT
N_CORES = 8


def _build(sim_mode=False, debug=False):
    nc = bacc.Bacc("TRN2", target_bir_lowering=False, debug=False,
                   num_devices=1 if sim_mode else N_CORES)

    x_d = nc.dram_tensor("x", [T, N, C], F32, kind="ExternalInput").ap()
    wq_d = nc.dram_tensor("wq", [2 * KT, 128, O3], F16, kind="ExternalInput").ap()
    wp_d = nc.dram_tensor("wp", [KT, 128, C], F16, kind="ExternalInput").ap()
    g1_d = nc.dram_tensor("g1", [128, OT1 * T], F32, kind="ExternalInput").ap()
    b1_d = nc.dram_tensor("b1", [128, OT1 * T], F32, kind="ExternalInput").ap()
    g2_d = nc.dram_tensor("g2", [128, OT2 * T], F32, kind="ExternalInput").ap()
    b2_d = nc.dram_tensor("b2", [128, OT2 * T], F32, kind="ExternalInput").ap()
    out_d = nc.dram_tensor("out", [T, N, C], F32, kind="ExternalOutput").ap()
    if debug:
        dbg_m = nc.dram_tensor("dbg_m", [128, KT * N], F16, kind="ExternalOutput").ap()
        dbg_y = nc.dram_tensor("dbg_y", [128, OT1 * N], F32, kind="ExternalOutput").ap()
        dbg_sc = nc.dram_tensor("dbg_sc", [128, OT1 * T], F32, kind="ExternalOutput").ap()
        dbg_bi = nc.dram_tensor("dbg_bi", [128, OT1 * T], F32, kind="ExternalOutput").ap()
        dbg_st = nc.dram_tensor("dbg_st", [128, 6 * OT1 * T], F32, kind="ExternalOutput").ap()
        dbg_yn = nc.dram_tensor("dbg_yn", [128, OT1 * N], F32, kind="ExternalOutput").ap()
        dbg_sT = nc.dram_tensor("dbg_sT", [128, OT1 * N], F16, kind="ExternalOutput").ap()
        dbg_att = nc.dram_tensor("dbg_att", [128, OT2 * N], F16, kind="ExternalOutput").ap()
        dbg_pot = nc.dram_tensor("dbg_pot", [128, OT2 * N], F32, kind="ExternalOutput").ap()

    with tile.TileContext(nc) as tc:
        import contextlib
        stack = contextlib.ExitStack()
        const = stack.enter_context(tc.tile_pool(name="const", bufs=1))
        state = stack.enter_context(tc.tile_pool(name="state", bufs=1))
        work = stack.enter_context(tc.tile_pool(name="work", bufs=2))
        ypool = stack.enter_context(tc.tile_pool(name="ypool", bufs=5))
        popool = stack.enter_context(tc.tile_pool(name="popool", bufs=4))
        psum = stack.enter_context(tc.tile_pool(name="psum", bufs=1, space="PSUM"))
        psum3 = stack.enter_context(tc.tile_pool(name="psum3", bufs=3, space="PSUM"))
        dram = stack.enter_context(tc.tile_pool(name="dram", bufs=1, space="DRAM"))

        def mmA():
            return psum3.tile([128, 2, 256], F32, tag="mmA", name="mmA")

        def o2d():
            return psum.tile([128, 512], F32, tag="o2d", name="o2d", bufs=2)

        ident = const.tile([128, 128], F16, tag="id16", name="ident")
        masks.make_identity(nc, ident[:])
        identf = const.tile([128, 128], F32, tag="id32", name="identf")
        masks.make_identity(nc, identf[:])

        nsl = [(0, NT0), (NT0, NT1)]

        # first x tiles load BEFORE weights: the SP queue + HWDGE serialize,
        # and LIF1(t=0) needs x immediately while matmuls need weights ~10us in
        xbufs = {}

        def load_x(t, eng=None):
            xs = [work.tile([128, C], F32, tag=f"x{i}", name=f"x{i}", bufs=3)
                  for i in range(2)]
            for i, (o, sz) in enumerate(nsl):
                (eng or nc.sync).dma_start(xs[i][:sz, :], x_d[t, o:o + sz, :])
            return xs

        xbufs[0] = load_x(0)
        xbufs[1] = load_x(1)

        wq = const.tile([128, 2 * KT, O3], F16, tag="wq", name="wq")
        wp = const.tile([128, KT, C], F16, tag="wp", name="wp")
        for k in range(2 * KT):
            nc.scalar.dma_start(wq[:, k, :], wq_d[k])
            if k < KT:
                nc.scalar.dma_start(wp[:, k, :], wp_d[k])

        g1 = const.tile([128, OT1 * T], F32, tag="g1", name="g1")
        b1 = const.tile([128, OT1 * T], F32, tag="b1", name="b1")
        g2 = const.tile([128, OT2 * T], F32, tag="g2", name="g2")
        b2 = const.tile([128, OT2 * T], F32, tag="b2", name="b2")
        for t_ap, d_ap in [(g1, g1_d), (b1, b1_d), (g2, g2_d), (b2, b2_d)]:
            nc.scalar.dma_start(t_ap[:], d_ap[:, :])

        # LIF state in natural units r = v_post: w = r + x (tt, 2x mode),
        # spike = w >= 2*VTH (ts, 4x), reset r = w * ((w < 2*VTH)*0.5)
        # (mask via ts 4x, apply via tt 2x) — no scalar_tensor_tensor
        # anywhere on the hot path (it has no DVE fast modes).
        vd1 = [state.tile([128, 2 * N], F32, tag=f"vd1_{i}", name=f"vd1_{i}")
               for i in range(2)]
        vd2 = state.tile([128, OT1 * N], F32, tag="vd2", name="vd2")
        vd4 = state.tile([128, OT2, N], F32, tag="vd4", name="vd4")

        # k/v spike transposes land here: cols 0:C = k^T, C:2C = v^T
        def kvT_t():
            return [work.tile([128, 2 * C], F16, tag=f"kvT{i}", name=f"kvT{i}",
                              bufs=2) for i in range(2)]
        # bn params (written per group)
        sc1 = state.tile([128, OT1 * T], F32, tag="sc1", name="sc1")
        bi1 = state.tile([128, OT1 * T], F32, tag="bi1", name="bi1")
        sc2 = state.tile([128, OT2 * T], F32, tag="sc2", name="sc2")
        bi2 = state.tile([128, OT2 * T], F32, tag="bi2", name="bi2")
        # raw bn_stats: 6 cols per (t, ot) = (c,m,M2)x2 INTERLEAVED halves
        # (even/odd element index) — merge is still sum/sumsq symmetric
        st1 = state.tile([128, 6 * OT1 * T], F32, tag="st1", name="st1")
        st2 = state.tile([128, 6 * OT2 * T], F32, tag="st2", name="st2")
        for s in (vd1[0], vd1[1], vd2, vd4):
            nc.gpsimd.memset(s[:], 0.0)

        # half-group (2-timestep) allreduce granularity
        NH = T // 2
        W1 = OT1 * 2
        W2 = OT2 * 2
        ar1_in = [dram.tile([128, 2 * W1], F32, tag=f"a1i{h}", name=f"a1i{h}") for h in range(NH)]
        ar1_out = [dram.tile([128, 2 * W1], F32, tag=f"a1o{h}", name=f"a1o{h}") for h in range(NH)]
        ar2_in = [dram.tile([128, 2 * W2], F32, tag=f"a2i{h}", name=f"a2i{h}") for h in range(NH)]
        ar2_out = [dram.tile([128, 2 * W2], F32, tag=f"a2o{h}", name=f"a2o{h}") for h in range(NH)]
        # last half-group: per-single-timestep AR2 windows (tail latency)
        ar2s_in = {t: dram.tile([128, 2 * OT2], F32, tag=f"a2si{t}", name=f"a2si{t}")
                   for t in (T - 2, T - 1)}
        ar2s_out = {t: dram.tile([128, 2 * OT2], F32, tag=f"a2so{t}", name=f"a2so{t}")
                    for t in (T - 2, T - 1)}


        # ---------------- phase A ----------------
        def do_A(t):
            xs = xbufs[t]

            # transpose x into [c, n] layout (fp32 PE transpose, two
            # psum half-fills), then LIF1 entirely on DVE in that layout.
            # The LIF mask m = (w < 2VTH)*0.5 doubles as BOTH the hard-reset
            # multiplier AND the matmul operand: the matmul consumes
            # m = (1-spike)/2 instead of the spike, which BN absorbs
            # (shift-invariant; host folds g -> -g, eps -> eps/4).
            m1t = work.tile([128, KT, N], F16, tag="m1t", name="m1t")
            for hf in range(2):
                tpx = psum.tile([128, 2, N], F32, tag="tpx", name="tpx")
                for ci in range(2):
                    ct = 2 * hf + ci
                    for i, (o, sz) in enumerate(nsl):
                        nc.tensor.transpose(tpx[:, ci, o:o + sz],
                                            xs[i][:sz, ct * 128:(ct + 1) * 128],
                                            identf[:sz, :sz])
                vsl = vd1[hf]
                w1 = work.tile([128, 2 * N], F32, tag=f"l1w{hf}",
                               name=f"l1w{hf}", bufs=1)
                nc.vector.scalar_tensor_tensor(w1[:, :], vsl[:, :], 0.5,
                                               tpx[:, :, :], op.mult, op.add)
                nc.vector.tensor_scalar(m1t[:, 2 * hf:2 * hf + 2, :], w1[:, :],
                                        1.0, 0.5, op.is_lt, op.mult)
                nc.vector.scalar_tensor_tensor(vsl[:, :], w1[:, :], 1.0,
                                               w1[:, :], op.is_lt, op.mult)

            # qkv matmuls: fp16 weights, 4 k-tiles accumulate per ot;
            # 4 ot per double-bank fill, one act evac + two bn_stats each
            yt = ypool.tile([128, OT1, N], F32, tag="y", name=f"y{t}")
            for bk in range(OT1 // 2):
                pA = mmA()
                for q in range(2):
                    ot = 2 * bk + q
                    for k in range(2 * KT):
                        nc.tensor.matmul(pA[:, q, 0:N],
                                         wq[:, k, ot * 128:(ot + 1) * 128],
                                         m1t[:, k % KT, :],
                                         start=(k == 0), stop=(k == 2 * KT - 1))
                ysl = yt[:, 2 * bk:2 * bk + 2, :]
                nc.scalar.activation(ysl, pA[:, :, 0:N], act.Copy)
                for q in range(2):
                    ot = 2 * bk + q
                    scol = (t * OT1 + ot) * 6
                    nc.vector.bn_stats(st1[:, scol:scol + 6], yt[:, ot, :])
            if debug and t == 0:
                nc.sync.dma_start(dbg_m[:, :], m1t[:, :, :])
                nc.sync.dma_start(dbg_y[:, :], yt[:, :, :])
            return yt

        # ---------------- collectives + params ----------------
        def stage_stats(st, h2, w, stg, tmpw):
            # merge the two interleaved halves per (t,ot):
            # sum = h*(m0+m1) ; sumsq = M2_0+M2_1 + h*(m0^2+m1^2), h = N/2
            base = h2 * w * 6
            end = (h2 + 1) * w * 6
            m0 = st[:, base + 1: end: 6]
            M20 = st[:, base + 2: end: 6]
            m1 = st[:, base + 4: end: 6]
            M21 = st[:, base + 5: end: 6]
            h = float(N // 2)
            nc.vector.tensor_tensor(tmpw[:, 0:w], m0, m1, op.add)
            nc.vector.tensor_scalar(stg[:, 0:w], tmpw[:, 0:w], h, None, op.mult)
            nc.vector.tensor_tensor(tmpw[:, 0:w], m0, m0, op.mult)
            nc.vector.tensor_tensor(tmpw[:, w:2 * w], m1, m1, op.mult)
            nc.vector.tensor_tensor(tmpw[:, 0:w], tmpw[:, 0:w], tmpw[:, w:2 * w], op.add)
            nc.vector.tensor_scalar(tmpw[:, 0:w], tmpw[:, 0:w], h, None, op.mult)
            nc.vector.tensor_tensor(tmpw[:, 0:w], tmpw[:, 0:w], M20, op.add)
            nc.vector.tensor_tensor(stg[:, w:2 * w], tmpw[:, 0:w], M21, op.add)

        def ar(h2, st, w, arin, arout):
            stg = const.tile([128, 2 * w], F32, tag=f"stg{w}", name=f"stg{w}", bufs=2)
            tmpw = const.tile([128, 2 * w], F32, tag=f"stgt{w}", name=f"stgt{w}", bufs=2)
            stage_stats(st, h2, w, stg, tmpw)
            nc.sync.dma_start(arin[h2][:, :], stg[:, :])
            if sim_mode:
                nc.sync.dma_start(arout[h2][:], arin[h2][:])
            else:
                nc.gpsimd.collective_compute(
                    "AllReduce", op.add,
                    ins=[arin[h2].opt()], outs=[arout[h2].opt()],
                    replica_groups=[list(range(N_CORES))])

        def params(h2, w, arout, g_t, b_t, sc, bi, pfx):
            # w cols; arout: sums [0:w], sumsq [w:2w]
            gsum = const.tile([128, 2 * w], F32, tag=f"{pfx}gs", name=f"{pfx}gs", bufs=2)
            nc.sync.dma_start(gsum[:], arout[h2][:])
            cs = slice(h2 * w, (h2 + 1) * w)
            mean = const.tile([128, w], F32, tag=f"{pfx}mu", name=f"{pfx}mu", bufs=2)
            e2p = const.tile([128, w], F32, tag=f"{pfx}e2", name=f"{pfx}e2", bufs=2)
            rs = const.tile([128, w], F32, tag=f"{pfx}rs", name=f"{pfx}rs", bufs=2)
            tmp = const.tile([128, w], F32, tag=f"{pfx}t1", name=f"{pfx}t1", bufs=2)
            tmp2 = const.tile([128, w], F32, tag=f"{pfx}t2", name=f"{pfx}t2", bufs=2)
            # stats are of y' = W^T m with m = (1-s)/2; y = c - 2y' means
            # var_y = 4 var' — handled by eps/4 + host-negated gammas.
            nc.vector.tensor_scalar(mean[:], gsum[:, 0:w], 1.0 / NB, None, op.mult)
            nc.vector.tensor_scalar(tmp[:], gsum[:, w:2 * w], 1.0 / NB, EPS / 4.0,
                                    op.mult, op.add)
            nc.vector.tensor_tensor(tmp2[:], mean[:], mean[:], op.mult)
            nc.vector.tensor_tensor(e2p[:], tmp[:], tmp2[:], op.subtract)  # var+eps
            nc.vector.reciprocal(tmp[:], e2p[:])
            nc.scalar.activation(rs[:], tmp[:], act.Sqrt)
            for _ in range(1):  # Newton: rs *= 1.5 - 0.5*(var+eps)*rs^2
                nc.vector.tensor_tensor(tmp[:], rs[:], rs[:], op.mult)
                nc.vector.tensor_tensor(tmp2[:], tmp[:], e2p[:], op.mult)
                nc.vector.tensor_scalar(tmp[:], tmp2[:], -0.5, 1.5, op.mult, op.add)
                nc.vector.tensor_tensor(rs[:], rs[:], tmp[:], op.mult)
            nc.vector.tensor_tensor(sc[:, cs], rs[:], g_t[:, cs], op.mult)
            nc.vector.tensor_tensor(tmp[:], mean[:], sc[:, cs], op.mult)
            nc.vector.tensor_tensor(bi[:, cs], b_t[:, cs], tmp[:], op.subtract)

        # ---------------- phase B ----------------
        def do_B_yn(t):
            # BN1 apply on Pool (per-partition scale+bias pointers),
            # prefetched one slot ahead of the LIF2 consumers
            yt = ybufs[t]
            yn = work.tile([128, OT1, N], F32, tag="yn", name=f"yn{t}", bufs=3)
            for q3 in (1, 2, 0):
                for oi in range(4):
                    ot = q3 * 4 + oi
                    col = t * OT1 + ot
                    eng = nc.scalar if q3 == 1 else nc.gpsimd
                    if True:
                        nc.scalar.activation(yn[:, ot, :], yt[:, ot, :],
                                             act.Identity,
                                             bias=bi1[:, col:col + 1],
                                             scale=sc1[:, col:col + 1])
                    else:
                        nc.gpsimd.tensor_scalar(yn[:, ot, :], yt[:, ot, :],
                                                sc1[:, col:col + 1],
                                                bi1[:, col:col + 1],
                                                op.mult, op.add)
            return yn

        def do_B(t):
            yn = ynbufs[t]
            sT = work.tile([128, OT1 * N], F16, tag="sT", name="sT")
            for q3 in (1, 2, 0):  # k/v first: attention deps resolve early
                # LIF2 on DVE: w = r + yn (tt 2x); spike/mask ts 4x; reset tt
                sl = slice(q3 * 4 * N, (q3 + 1) * 4 * N)
                ysl = yn[:, q3 * 4:(q3 + 1) * 4, :]
                w2 = work.tile([128, 4 * N], F32, tag="w2B", name="w2B", bufs=2)
                nc.vector.scalar_tensor_tensor(w2[:], vd2[:, sl], 0.5,
                                               ysl, op.mult, op.add)
                nc.gpsimd.tensor_scalar(sT[:, sl], w2[:], 1.0, None, op.is_ge)
                nc.vector.scalar_tensor_tensor(vd2[:, sl], w2[:], 1.0,
                                               w2[:], op.is_lt, op.mult)

            if debug and t == 0:
                nc.sync.dma_start(dbg_yn[:, :], ynbufs[t][:, :, :])
                nc.sync.dma_start(dbg_sT[:, :], sT[:, :])

            # attention: k/v transposes into one psum bank per n-slice
            kvT = kvT_t()
            for i, (o, sz) in enumerate(nsl):
                tp2 = psum.tile([128, 2, C], F16, tag="tp2", name="tp2")
                for j in (1, 2):  # k -> cols 0:C, v -> cols C:2C
                    for ci in range(4):
                        otg = 4 * j + ci
                        nc.tensor.transpose(tp2[:sz, j - 1, ci * 128:(ci + 1) * 128],
                                            sT[:, otg * N + o: otg * N + o + sz],
                                            ident[:128, :128])
                nc.scalar.activation(kvT[i][:sz, :], tp2[:sz, :, :], act.Copy)

            kvp = o2d()[:, 0:256]
            for ct in range(4):
                for hh in range(2):
                    h = 2 * ct + hh
                    off = hh * 64
                    hc = h * 64
                    nc.tensor.matmul(kvp[off:off + 64, ct * 64:(ct + 1) * 64],
                                     kvT[0][:, hc:hc + 64],
                                     kvT[0][:, C + hc:C + hc + 64],
                                     start=True, stop=False,
                                     tile_position=(0, off))
                    nc.tensor.matmul(kvp[off:off + 64, ct * 64:(ct + 1) * 64],
                                     kvT[1][:NT1, hc:hc + 64],
                                     kvT[1][:NT1, C + hc:C + hc + 64],
                                     start=False, stop=True,
                                     tile_position=(0, off))
            kv = work.tile([128, 256], F16, tag="kv", name="kv", bufs=1)
            nc.scalar.activation(kv[:, :], kvp[:, :], act.Copy)

            # q@kv into psum, evac to att f16 (integer counts: exact)
            att = work.tile([128, OT2, N], F16, tag="att", name="att", bufs=1)
            for cp in range(2):
                outp = o2d()[:, 0:2 * N]
                for q in range(2):
                    ct = 2 * cp + q
                    for hh in range(2):
                        off = hh * 64
                        nc.tensor.matmul(outp[off:off + 64, q * N:(q + 1) * N],
                                         kv[off:off + 64, ct * 64:(ct + 1) * 64],
                                         sT[off:off + 64, ct * N:(ct + 1) * N],
                                         start=True, stop=True,
                                         tile_position=(off, off))
                nc.scalar.activation(att[:, 2 * cp:2 * cp + 2, :], outp, act.Copy)

            # LIF-proj on DVE (state x8: w = r + att, threshold 8); the mask
            # m4 = (w<8)*0.5 is both the reset multiplier and the proj
            # matmul operand (BN2 absorbs the 1-2m substitution)
            w4 = work.tile([128, OT2, N], F32, tag="w4", name="w4", bufs=1)
            m4 = work.tile([128, KT, N], F16, tag="m4", name="m4", bufs=2)
            nc.vector.scalar_tensor_tensor(w4[:, :, :], vd4[:, :, :], 0.5,
                                           att[:, :, :], op.mult, op.add)
            nc.vector.tensor_scalar(m4[:, :, :], w4[:, :, :], 8.0, 0.5,
                                    op.is_lt, op.mult)
            nc.vector.scalar_tensor_tensor(vd4[:, :, :], w4[:, :, :], 8.0,
                                           w4[:, :, :], op.is_lt, op.mult)

            # proj matmuls: all 4 ot in one double-bank fill
            pot = popool.tile([128, OT2, N], F32, tag="po", name=f"po{t}")
            for bk in range(OT2 // 2):
                pP = mmA()
                for q in range(2):
                    ot = 2 * bk + q
                    for k in range(KT):
                        nc.tensor.matmul(pP[:, q, 0:N],
                                         wp[:, k, ot * 128:(ot + 1) * 128],
                                         m4[:, k, :],
                                         start=(k == 0), stop=(k == KT - 1))
                nc.scalar.activation(pot[:, 2 * bk:2 * bk + 2, :], pP[:, :, 0:N],
                                     act.Copy)
            for ot in range(OT2):
                scol = (t * OT2 + ot) * 6
                nc.vector.bn_stats(st2[:, scol:scol + 6], pot[:, ot, :])
            if debug and t == 0:
                nc.sync.dma_start(dbg_att[:, :], att[:, :, :])
                nc.sync.dma_start(dbg_pot[:, :], pot[:, :, :])
            return pot

        # ---------------- phase C ----------------
        def do_C(t):
            pot = pobufs[t]
            fin = work.tile([128, OT2, N], F32, tag="fin", name="fin")
            for ot in range(OT2):
                col = t * OT2 + ot
                nc.gpsimd.tensor_scalar(fin[:, ot, :], pot[:, ot, :],
                                        sc2[:, col:col + 1],
                                        bi2[:, col:col + 1],
                                        op.mult, op.add)
            for i, (o, sz) in enumerate(nsl):
                tpf = psum.tile([128, C], F32, tag="ftp", name="ftp")
                for ot in range(OT2):
                    nc.tensor.transpose(tpf[:sz, ot * 128:(ot + 1) * 128],
                                        fin[:, ot, o:o + sz],
                                        identf[:128, :128])
                fout = work.tile([128, C], F32, tag=f"fo{i}", name=f"fo{i}", bufs=2)
                nc.scalar.activation(fout[:sz, :], tpf[:sz, :], act.Copy)
                nc.sync.dma_start(out_d[t, o:o + sz, :], fout[:sz, :])

        # ---------------- pipelined emission (flat slot schedule) ----------
        # per timestep-slot s: A(s); ar1 after A(2h+1); params1 one slot on;
        # yn(t) prefetched at t+YL; B(t) at t+BL; ar2 right after B(2h+1);
        # params2 next slot; C(t) at t+CL.
        # B-work and yn are enqueued BEFORE A(s) so the latency-critical
        # attention/LIF chains sit ahead of A's bulk in the engine FIFOs;
        # x DMA is prefetched one slot ahead.
        BL, CL = 8, 10
        YL = BL - 2
        ybufs = {}
        ynbufs = {}
        pobufs = {}
        xbufs[0] = load_x(0)
        for s in range(T + CL + 1):
            if 2 <= s + 1 < T:
                xbufs[s + 1] = load_x(s + 1)

            if s >= 2 and (s - 2) % 2 == 0 and (s - 2) // 2 < NH:
                ar((s - 2) // 2, st1, W1, ar1_in, ar1_out)
            if s >= 3 and (s - 3) % 2 == 0 and (s - 3) // 2 < NH:
                params((s - 3) // 2, W1, ar1_out, g1, b1, sc1, bi1, "p1")
            if 0 <= s - YL < T:
                ynbufs[s - YL] = do_B_yn(s - YL)
            if 0 <= s - BL < T:
                pobufs[s - BL] = do_B(s - BL)
            hs = s - BL
            if hs >= 1 and (hs - 1) % 2 == 0 and (hs - 1) // 2 < NH - 1:
                ar((hs - 1) // 2, st2, W2, ar2_in, ar2_out)
            if hs >= 2 and (hs - 2) % 2 == 0 and (hs - 2) // 2 < NH - 1:
                params((hs - 2) // 2, W2, ar2_out, g2, b2, sc2, bi2, "p2")
            # last two timesteps: single-t AR2 right after each B
            if hs in (T - 2, T - 1):
                ar(hs, st2, OT2, ar2s_in, ar2s_out)
            if hs - 1 in (T - 2, T - 1):
                params(hs - 1, OT2, ar2s_out, g2, b2, sc2, bi2, "p2s")
            if s < T:
                ybufs[s] = do_A(s)
            if 0 <= s - CL < T:
                do_C(s - CL)

        if debug:
            nc.sync.dma_start(dbg_sc[:, :], sc1[:, :])
            nc.sync.dma_start(dbg_bi[:, :], bi1[:, :])
            nc.sync.dma_start(dbg_st[:, :], st1[:, :])

        stack.close()

    nc.compile()
    return nc


# ---------------- host-side prep ----------------

def _bn_layout(v, Tn, OT):
    return np.ascontiguousarray(
        np.asarray(v, np.float32).reshape(Tn, OT, 128)
        .transpose(2, 0, 1).reshape(128, OT * Tn))


def _prep(inputs):
    qkv_w = np.asarray(inputs["qkv_w"], dtype=np.float32)
    proj_w = np.asarray(inputs["proj_w"], dtype=np.float32)
    w1t = np.ascontiguousarray(qkv_w.T)   # [512, 1536]
    w2t = np.ascontiguousarray(proj_w.T)  # [512, 512]
    def hilo(w, M):
        hi = w.astype(np.float16)
        lo = (w - hi.astype(np.float32)).astype(np.float16)
        return np.concatenate([hi.reshape(KT, 128, M), lo.reshape(KT, 128, M)], axis=0)
    wq = hilo(w1t, O3)
    # proj has no downstream thresholds: its quantization error is smooth
    # (no spike-flip cascade), so a single fp16 plane suffices (L2 ~2.6e-4)
    wp = w2t.reshape(KT, 128, C).astype(np.float16)

    # matmuls consume masks m = (1-spike)/2, so y = c - 2*y' and BN needs
    # sc_eff = -g * rsqrt(var'+eps/4), bi_eff = b - mean' * sc_eff: negate g.
    g1 = -_bn_layout(inputs["bn1_g"], T, OT1)
    b1 = _bn_layout(inputs["bn1_b"], T, OT1)
    g2 = -_bn_layout(inputs["bn2_g"], T, OT2)
    b2 = _bn_layout(inputs["bn2_b"], T, OT2)
    return dict(wq=wq, wp=wp, g1=g1, b1=b1, g2=g2, b2=b2)


_CACHE = {}


def kernel(_trace=False, **inputs):
    for k in ("w_in", "w_q", "w_k", "w_v", "w_proj"):
        assert float(np.asarray(inputs[k])) == 0.0, "kernel assumes sigmoid(w)=0.5"
    if "nc" not in _CACHE:
        _CACHE["nc"] = _build()
    nc = _CACHE["nc"]

    shared = _prep(inputs)
    x = np.asarray(inputs["x"], dtype=np.float32)
    in_maps = []
    for b in range(N_CORES):
        m = dict(shared)
        m["x"] = np.ascontiguousarray(x[:, b])
        in_maps.append(m)

    res = bass_utils.run_bass_kernel_spmd(nc, in_maps, core_ids=list(range(N_CORES)),
                                          trace=_trace)
    out = np.stack([r["out"] for r in res.results], axis=1)
    if _trace:
        return out, res
    return out
